# revision 1
# baseline (speedup 1.0000x reference)
"""Causal self-attention block (QKV proj + RoPE + causal attention + o_proj)
on 8 Trainium2 NeuronCores.

Sharding: tensor-parallel over heads for QKV+attention (core c owns head
indices 4c..4c+3 of BOTH batch elements). The attention outputs are written
in an 8-shard layout; the host permutes shards (a pure gather, the moral
equivalent of an AllToAll - on-device collectives fail to load under this
runtime) so that launch 2 runs o_proj token-sharded: core c handles
(batch c//4, token block c%4) with the full o_proj weight. The final host
step is a pure concatenation of the 8 disjoint output slices.

Device notes:
- All GEMM operands float16 (1 PE cycle/row; fp32 PSUM accumulation).
- q/k/v stay SBUF-resident between the QKV phase and the attention phase
  (no DRAM round-trip, no per-head reloads, no phase-transition bubble).
- All DRAM operands are host-preblocked so DMA lands as long contiguous
  per-partition runs: the HWDGE is descriptor-rate-bound (~45ns/desc) on
  transposed access patterns, which used to cost ~25us of launch head.
- Softmax is max-free: Exp on ScalarE straight out of PSUM, skipping
  fully-masked column ranges of diagonal pairs. The denominators are
  accumulated per-pair on VectorE (two interleaved partial accumulators)
  and reduced with one ones-vector matmul pair per query tile instead of
  one per key chunk - removing ~100us of PE time vs summing in the PV
  loop. The normalization tail is deferred and split in two stages
  emitted after the NEXT tile's scores pairs, so the in-order PE never
  waits on VectorE; lps/rps share the aps PSUM tag (the accumulator is
  evicted early to SBUF, alternating ScalarE/VectorE to balance load),
  freeing enough PSUM for a 3-deep score pipeline.
- Engine balance in attention: PE ~182us, ScalarE ~178us, VectorE
  ~176us over a ~193us phase - all three near-saturated.
"""

import numpy as np

import concourse.bass as bass
import concourse.bass_isa as bass_isa
import concourse.tile as tile
from concourse import bacc, mybir
from concourse.bass_utils import run_bass_kernel_spmd

HIDDEN = 4096
N_HEADS = 32
HEAD_DIM = 128
B, S = 2, 2048
T = B * S
N_CORES = 8
HPC = N_HEADS // N_CORES  # 4 head indices per core (both batches)
ROPE_BASE = 10000.0

FP = mybir.dt.float16
F32 = mybir.dt.float32

M_B = 512                # token block in QKV GEMM
NBLK = T // M_B          # 8 blocks
KC = HIDDEN // 128       # 32 contraction chunks
TQ = 512                 # query tile in attention
SCHUNKS = S // 128       # 16 key chunks per batch


def build_nc1():
    """QKV projection + RoPE + causal attention. Output: attention results
    in 8-shard layout attnT[s, :, :] = shard for (batch s//4, tokblock s%4),
    rows = this core's 4 heads x 128 dims, cols = 512 tokens (fp16)."""
    nc = bacc.Bacc(num_devices=N_CORES, trn_type="TRN2")

    # host-preblocked so every DMA lands as long contiguous per-partition
    # runs (the HWDGE is descriptor-rate-bound on transposed loads)
    xT = nc.declare_dram_parameter("xT", [128, NBLK, KC, M_B], FP,
                                   isOutput=False)
    wqk = nc.declare_dram_parameter("wqk", [8, 128, KC, 128], FP, isOutput=False)
    wv = nc.declare_dram_parameter("wv", [128, KC, 512], FP, isOutput=False)
    cosq = nc.declare_dram_parameter("cosq", [128, T], FP, isOutput=False)
    sinq = nc.declare_dram_parameter("sinq", [128, T], FP, isOutput=False)
    cosk = nc.declare_dram_parameter("cosk", [128, T], FP, isOutput=False)
    sink = nc.declare_dram_parameter("sink", [128, T], FP, isOutput=False)
    masks = nc.declare_dram_parameter("masks", [4, 128, TQ], FP, isOutput=False)
    onesc = nc.declare_dram_parameter("onesc", [128, 1], FP, isOutput=False)
    onesr = nc.declare_dram_parameter("onesr", [1, 128], FP, isOutput=False)
    attnT = nc.declare_dram_parameter("attnT", [8, 512, 512], FP, isOutput=True)

    with nc.allow_low_precision(reason="float16 carries the data; fp32 PSUM"), \
         tile.TileContext(nc) as tc:
        with tc.tile_pool(name="res", bufs=1) as rpool_res:
            # q/k feature-major [head_dim, 4 heads, T]; v token-major
            # [128 tok, 32 chunks, 4 heads x 128 dims]. SBUF-resident.
            q_res = rpool_res.tile([128, HPC, T], FP, name="q_res")
            k_res = rpool_res.tile([128, HPC, T], FP, name="k_res")
            v_res = rpool_res.tile([128, T // 128, 512], FP, name="v_res")
            # attention constants (DMAs emitted after block 0's x loads so
            # they never delay the first matmuls)
            ones_col = rpool_res.tile([128, 1], FP, name="ones_col")
            ones_row = rpool_res.tile([1, 128], FP, name="ones_row")
            msk = rpool_res.tile([128, 2, 2, TQ], FP, name="msk")

            # ---------------- Phase A: QKV projection + RoPE ----------------
            with tc.tile_pool(name="xblk", bufs=1) as xpool, \
                 tc.tile_pool(name="wq", bufs=1) as wqpool, \
                 tc.tile_pool(name="wvp", bufs=1) as wvpool, \
                 tc.tile_pool(name="rope", bufs=1) as ropool, \
                 tc.tile_pool(name="tmpp", bufs=1) as tpool, \
                 tc.tile_pool(name="psA", bufs=1, space="PSUM") as psA:
                for blk in range(NBLK):
                    t0 = blk * M_B
                    xb = xpool.tile([128, KC, M_B], FP, tag="xb", bufs=2)
                    # chunked load: first v matmul starts after ~0.5MB lands
                    for g in range(8):
                        nc.sync.dma_start(
                            xb[:, g * 4:(g + 1) * 4, :],
                            xT[:, blk, g * 4:(g + 1) * 4, :])
                    if blk == 0:
                        nc.sync.dma_start(ones_col[:], onesc[:])
                        nc.sync.dma_start(ones_row[:], onesr[:])
                        nc.sync.dma_start(
                            msk[:],
                            masks.rearrange("(pi hf) p f -> p pi hf f", pi=2))

                    # v: token-major [tok 128, feat 512], accumulate over kc
                    vps = []
                    for tt in range(4):
                        vps.append(psA.tile([128, 512], F32, tag=f"vps{tt}",
                                            bufs=1, name=f"vps{tt}"))
                    for g in range(KC // 4):
                        wvt = wvpool.tile([128, 4, 512], FP, tag="wv", bufs=2)
                        nc.sync.dma_start(wvt[:], wv[:, g * 4:(g + 1) * 4, :])
                        for i in range(4):
                            kc = g * 4 + i
                            for tt in range(4):
                                nc.tensor.matmul(
                                    vps[tt][:],
                                    xb[:, kc, tt * 128:(tt + 1) * 128],
                                    wvt[:, i, :],
                                    start=(kc == 0), stop=(kc == KC - 1))
                    for tt in range(4):
                        nc.vector.tensor_copy(
                            v_res[:, blk * 4 + tt, :], vps[tt][:])

                    # q, k: feature-major [head_dim 128, tok 512] + fused RoPE
                    cq = ropool.tile([128, M_B], FP, tag="cq", bufs=2)
                    sq = ropool.tile([128, M_B], FP, tag="sq", bufs=2)
                    ck = ropool.tile([128, M_B], FP, tag="ck", bufs=2)
                    sk = ropool.tile([128, M_B], FP, tag="sk", bufs=2)
                    nc.sync.dma_start(cq[:], cosq[:, t0:t0 + M_B])
                    nc.sync.dma_start(sq[:], sinq[:, t0:t0 + M_B])
                    nc.sync.dma_start(ck[:], cosk[:, t0:t0 + M_B])
                    nc.sync.dma_start(sk[:], sink[:, t0:t0 + M_B])
                    for ft in range(8):
                        qp = psA.tile([128, M_B], F32, tag="qkps", bufs=2)
                        wt = wqpool.tile([128, KC, 128], FP, tag="wqk", bufs=2)
                        nc.sync.dma_start(wt[:], wqk[ft])
                        for kc in range(KC):
                            nc.tensor.matmul(qp[:], wt[:, kc, :], xb[:, kc, :],
                                             start=(kc == 0), stop=(kc == KC - 1))
                        is_q = ft < 4
                        h = ft % 4
                        cos2, sin2 = (cq, sq) if is_q else (ck, sk)
                        dst = (q_res if is_q else k_res)[:, h, t0:t0 + M_B]
                        tmp = tpool.tile([128, M_B], FP, tag="ropetmp", bufs=2)
                        nc.vector.tensor_mul(dst, qp[:], cos2[:])
                        nc.vector.tensor_mul(tmp[0:64, :], qp[64:128, :],
                                             sin2[0:64, :])
                        nc.vector.tensor_mul(tmp[64:128, :], qp[0:64, :],
                                             sin2[64:128, :])
                        nc.vector.tensor_sub(dst[0:64], dst[0:64], tmp[0:64, :])
                        nc.vector.tensor_add(dst[64:128], dst[64:128],
                                             tmp[64:128, :])

            # ---------------- Phase B: causal attention ----------------
            with tc.tile_pool(name="probs", bufs=4) as ppool, \
                 tc.tile_pool(name="accp", bufs=1) as apool, \
                 tc.tile_pool(name="yt", bufs=3) as ypool, \
                 tc.tile_pool(name="psB", bufs=1, space="PSUM") as psB:
                # the softmax-normalization tail of tile j is emitted in two
                # stages interleaved after tile j+1's scores pairs, so the
                # in-order PE never stalls waiting for the VectorE prob
                # accumulator (stage 1: lps) or the r-copy (stage 2: rps).
                # lps/rps share the "aps" PSUM tag: aps is freed early by the
                # araw eviction, so 3 allocations/tile rotate cleanly through
                # 2 banks, leaving 6 banks for a 3-deep score pipeline.
                pending_tail = []

                def flush_one():
                    if pending_tail:
                        pending_tail.pop(0)()

                def flush_tail():
                    while pending_tail:
                        pending_tail.pop(0)()

                for bb in range(B):
                    for h in range(HPC):
                        for j in range(S // TQ):
                            nchunks = 4 * j + 4
                            npairs = nchunks // 2
                            aps_box = [None]
                            # prob accumulators: even pairs on VectorE, odd
                            # pairs on the otherwise-idle GpSimd engine; the
                            # lps matmul sums both partials in one PSUM group
                            acc_e = apool.tile([128, TQ], FP, tag="acce",
                                               bufs=2)
                            acc_o = apool.tile([128, TQ], FP, tag="acco",
                                               bufs=2)

                            def scores(p):
                                # two key chunks share one 2-bank PSUM tile so
                                # a single Exp covers 1024 columns
                                sps = psB.tile([128, 2, TQ], F32, tag="sps",
                                               bufs=3, name="sps")
                                for hf in range(2):
                                    c = 2 * p + hf
                                    nc.tensor.matmul(
                                        sps[:, hf, :],
                                        k_res[:, h, bb * S + c * 128:
                                              bb * S + (c + 1) * 128],
                                        q_res[:, h, bb * S + j * TQ:
                                              bb * S + (j + 1) * TQ],
                                        start=True, stop=True)
                                pr = ppool.tile([128, 2, TQ], FP, tag="pr",
                                                bufs=6, name="pr")
                                diag = 2 * p >= 4 * j
                                pi = p - 2 * j
                                # for the second diagonal pair (chunks s=2,3)
                                # queries 0..255 are fully masked: skip their
                                # Exp, zero them instead
                                e0 = 256 if (diag and pi == 1) else 0
                                nc.scalar.activation(
                                    pr[:, :, e0:TQ], sps[:, :, e0:TQ],
                                    mybir.ActivationFunctionType.Exp)
                                if e0:
                                    nc.vector.memset(pr[:, 0, 0:e0], 0.0)
                                    nc.vector.memset(pr[:, 1, 0:384], 0.0)
                                if diag:
                                    for hf in range(2):
                                        s = 2 * pi + hf
                                        # cols < 128s are fully masked: for
                                        # pi=1 the memsets above zero them;
                                        # for pi=0 the mask itself must
                                        # (mask[:, :128s] is 0)
                                        lo = max(e0, 128 * s) if e0 else 0
                                        w = 128 * (s + 1)
                                        nc.vector.tensor_mul(
                                            pr[:, hf, lo:w], pr[:, hf, lo:w],
                                            msk[:, pi, hf, lo:w])
                                acc = acc_e if p % 2 == 0 else acc_o
                                if p < 2:
                                    nc.vector.tensor_add(acc[:], pr[:, 0, :],
                                                         pr[:, 1, :])
                                else:
                                    nc.vector.tensor_add(acc[:], acc[:],
                                                         pr[:, 0, :])
                                    nc.vector.tensor_add(acc[:], acc[:],
                                                         pr[:, 1, :])
                                return pr

                            def pv(p, pr):
                                if aps_box[0] is None:
                                    aps_box[0] = psB.tile([128, TQ], F32,
                                                          tag="aps", bufs=2,
                                                          name="aps")
                                for hf in range(2):
                                    c = 2 * p + hf
                                    nc.tensor.matmul(
                                        aps_box[0][:],
                                        v_res[:, bb * SCHUNKS + c,
                                              h * 128:(h + 1) * 128],
                                        pr[:, hf, :],
                                        start=(c == 0), stop=(c == nchunks - 1))

                            # software pipeline: scores for pair p+1 issued
                            # before the PV matmuls of pair p, so the in-order
                            # PE has independent work while Exp(p) runs on ACT
                            prev = scores(0)
                            flush_one()  # prev tile's norm: PE-safe spots
                            for p in range(1, npairs):
                                cur = scores(p)
                                flush_one()
                                pv(p - 1, prev)
                                prev = cur
                            pv(npairs - 1, prev)
                            aps = aps_box[0]

                            # free the PSUM accumulator immediately with a
                            # fast DVE copy; normalization happens off the
                            # PE critical path once the denominators land
                            araw = ypool.tile([128, TQ], FP, tag="araw",
                                              bufs=4)
                            # alternate the eviction between ScalarE and
                            # VectorE to keep the two engines balanced
                            if j % 2 == 0:
                                nc.scalar.copy(araw[:], aps[:])
                            else:
                                nc.vector.tensor_copy(araw[:], aps[:])

                            def mk_tail(acc_e=acc_e, acc_o=acc_o, araw=araw,
                                        bb=bb, h=h, j=j):
                                box = {}

                                def stage1():
                                    lps = psB.tile([128, TQ], F32, tag="aps",
                                                   bufs=2, name="lps")
                                    nc.tensor.matmul(lps[0:1, :], ones_col[:],
                                                     acc_e[:], start=True,
                                                     stop=False)
                                    nc.tensor.matmul(lps[0:1, :], ones_col[:],
                                                     acc_o[:], start=False,
                                                     stop=True)
                                    r = ypool.tile([1, TQ], FP, tag="r",
                                                   bufs=2)
                                    nc.scalar.copy(r[:], lps[0:1, :])
                                    box["r"] = r

                                def stage2():
                                    rps = psB.tile([128, TQ], F32, tag="aps",
                                                   bufs=2, name="rps")
                                    nc.tensor.matmul(rps[:], ones_row[:],
                                                     box["r"][:],
                                                     start=True, stop=True)
                                    rrep = ypool.tile([128, TQ], F32,
                                                      tag="rrep", bufs=2)
                                    nc.vector.reciprocal_approx_fast(rrep[:],
                                                                     rps[:])
                                    yt = ypool.tile([128, TQ], FP, tag="yt")
                                    nc.vector.tensor_mul(yt[:], araw[:],
                                                         rrep[:])
                                    nc.sync.dma_start(
                                        attnT[bb * 4 + j,
                                              h * 128:(h + 1) * 128, :],
                                        yt[:])
                                return [stage1, stage2]

                            pending_tail.extend(mk_tail())
                flush_tail()

    nc.finalize()
    return nc


def build_nc2():
    """o_proj: out[of, tok] = sum_f WoT[f, of] * yT[f, tok] for this core's
    (batch, token-block) slice, with the full o_proj weight."""
    nc = bacc.Bacc(num_devices=N_CORES, trn_type="TRN2")
    yT = nc.declare_dram_parameter("yT", [128, KC, 512], FP, isOutput=False)
    wo = nc.declare_dram_parameter("wo", [32, 128, KC, 128], FP, isOutput=False)
    out = nc.declare_dram_parameter("out", [HIDDEN, 512], FP, isOutput=True)

    with nc.allow_low_precision(reason="float16 output; fp32 PSUM"), \
         tile.TileContext(nc) as tc:
        with tc.tile_pool(name="yblk", bufs=1) as ybpool, \
             tc.tile_pool(name="wop", bufs=4) as wopool, \
             tc.tile_pool(name="oev", bufs=3) as oepool, \
             tc.tile_pool(name="psD", bufs=1, space="PSUM") as psD:
            # first weight tile before the activation block: the of=0 matmuls
            # gate on wt0 + the first ysb chunk only
            wts = []
            for pre in range(2):
                wt = wopool.tile([128, KC, 128], FP, tag="wo", bufs=3)
                nc.sync.dma_start(wt[:], wo[pre])
                wts.append(wt)
            ysb = ybpool.tile([128, KC, 512], FP)
            for kq in range(8):
                nc.sync.dma_start(
                    ysb[:, kq * 4:(kq + 1) * 4, :],
                    yT[:, kq * 4:(kq + 1) * 4, :])
            for of in range(32):
                ops = psD.tile([128, 512], F32, tag="ops", bufs=4)
                if of < 2:
                    wt = wts[of]
                else:
                    wt = wopool.tile([128, KC, 128], FP, tag="wo", bufs=3)
                    nc.sync.dma_start(wt[:], wo[of])
                for kc in range(KC):
                    nc.tensor.matmul(ops[:], wt[:, kc, :], ysb[:, kc, :],
                                     start=(kc == 0), stop=(kc == KC - 1))
                osb = oepool.tile([128, 512], FP, tag="oev")
                nc.vector.tensor_copy(osb[:], ops[:])
                nc.sync.dma_start(out[of * 128:(of + 1) * 128, :], osb[:])

    nc.finalize()
    return nc


_NC1 = None
_NC2 = None


def get_ncs():
    global _NC1, _NC2
    if _NC1 is None:
        _NC1 = build_nc1()
        _NC2 = build_nc2()
    return _NC1, _NC2


def _rope_tables(positions):
    """positions [B, S] int -> packed cos/sin tables [128, T] f32 in token
    order (b*S + t); rows [0:64] and [64:128] hold the same 64 freqs."""
    inv_freq = 1.0 / (ROPE_BASE ** (np.arange(0, HEAD_DIM, 2, dtype=np.float64)
                                    / HEAD_DIM))
    freqs = np.asarray(positions).reshape(T).astype(np.float64)[:, None] * inv_freq
    cos = np.cos(freqs).T.astype(np.float32)  # [64, T]
    sin = np.sin(freqs).T.astype(np.float32)
    cos2 = np.concatenate([cos, cos], axis=0)  # [128, T]
    sin2 = np.concatenate([sin, sin], axis=0)
    scale = np.float32(HEAD_DIM ** -0.5)
    return ((cos2 * scale).astype(np.float16), (sin2 * scale).astype(np.float16),
            cos2.astype(np.float16), sin2.astype(np.float16))


def prepare_inputs1(hidden_states, positions, W_pack):
    x = np.ascontiguousarray(np.asarray(hidden_states, dtype=np.float32)
                             .reshape(T, HIDDEN))
    # [p, blk, kc, t]: per-partition-contiguous blocks (cheap DMA descriptors)
    xT_blocks = np.ascontiguousarray(
        x.T.astype(np.float16).reshape(KC, 128, NBLK, M_B)
        .transpose(1, 2, 0, 3))

    cosq, sinq, cosk, sink = _rope_tables(positions)

    mk = np.zeros((4, 128, TQ), dtype=np.float16)
    p = np.arange(128)[:, None]
    f = np.arange(TQ)[None, :]
    for d in range(4):
        mk[d] = (p + 128 * d <= f).astype(np.float16)

    in_maps = []
    for c in range(N_CORES):
        hs = [HPC * c + i for i in range(HPC)]
        wqk_blocks = np.empty((8, 128, KC, 128), dtype=np.float16)
        for ft in range(8):
            off = 0 if ft < 4 else HIDDEN
            h = hs[ft % 4]
            wsl = W_pack[off + h * 128: off + (h + 1) * 128, :]  # [128, 4096]
            wqk_blocks[ft] = wsl.reshape(128, KC, 128).transpose(2, 1, 0)
        wv_sl = np.concatenate(
            [W_pack[2 * HIDDEN + h * 128: 2 * HIDDEN + (h + 1) * 128, :]
             for h in hs], axis=0)  # [512, 4096]
        wv_blocks = np.ascontiguousarray(
            wv_sl.astype(np.float16).reshape(512, KC, 128).transpose(2, 1, 0))
        in_maps.append({
            "xT": xT_blocks,
            "wqk": np.ascontiguousarray(wqk_blocks),
            "wv": wv_blocks,
            "cosq": cosq, "sinq": sinq, "cosk": cosk, "sink": sink,
            "masks": mk,
            "onesc": np.ones((128, 1), dtype=np.float16),
            "onesr": np.ones((1, 128), dtype=np.float16),
        })
    return in_maps


def prepare_inputs2(res1, W_o):
    """Host-side shard permutation (the "AllToAll"): pure gather, no math."""
    wo_blocks = np.ascontiguousarray(
        np.ascontiguousarray(np.asarray(W_o, dtype=np.float32).T
                             .astype(np.float16))
        .reshape(KC, 128, 32, 128).transpose(2, 1, 0, 3))
    in_maps2 = []
    for j in range(N_CORES):
        yT = np.concatenate([res1.results[c]["attnT"][j] for c in range(N_CORES)],
                            axis=0)  # [4096, 512] feature-major, head order
        yTp = np.ascontiguousarray(
            yT.reshape(KC, 128, 512).transpose(1, 0, 2))  # [p, kc, t]
        in_maps2.append({"yT": yTp, "wo": wo_blocks})
    return in_maps2


def assemble(res2):
    out = np.empty((B, S, HIDDEN), dtype=np.float32)
    for c in range(N_CORES):
        bb, j = c // 4, c % 4
        out[bb, j * 512:(j + 1) * 512, :] = \
            res2.results[c]["out"].T.astype(np.float32)
    return out


def run(hidden_states, positions, W_pack, W_o, trace=False):
    nc1, nc2 = get_ncs()
    in_maps1 = prepare_inputs1(hidden_states, positions,
                               np.asarray(W_pack, dtype=np.float32))
    res1 = run_bass_kernel_spmd(nc1, in_maps1, list(range(N_CORES)),
                                trace=trace)
    in_maps2 = prepare_inputs2(res1, W_o)
    res2 = run_bass_kernel_spmd(nc2, in_maps2, list(range(N_CORES)),
                                trace=trace)
    return assemble(res2), res1, res2


def kernel(hidden_states, positions, W_pack, W_o):
    out, _, _ = run(hidden_states, positions, W_pack, W_o)
    return out



# revision 13
# speedup vs baseline: 1.0087x; 1.0087x over previous
"""Causal self-attention block (QKV proj + RoPE + causal attention + o_proj)
on 8 Trainium2 NeuronCores.

Sharding: tensor-parallel over heads for QKV+attention (core c owns head
indices 4c..4c+3 of BOTH batch elements). The attention outputs are written
in an 8-shard layout; the host permutes shards (a pure gather, the moral
equivalent of an AllToAll - on-device collectives fail to load under this
runtime) so that launch 2 runs o_proj token-sharded: core c handles
(batch c//4, token block c%4) with the full o_proj weight. The final host
step is a pure concatenation of the 8 disjoint output slices.

Launch-1 structure (the big change vs the phase-serial version): the
attention work for token block g is INTERLEAVED into token block g+1's QKV
GEMM emission. The QKV phase is pure PE work (~660us) with idle ScalarE/
VectorE; the attention phase alone is jointly PE+ScalarE+VectorE bound
(~205us span over ~160us of PE). Fused, the attention Exp/accumulate work
hides completely under the QKV matmul stream and launch 1 approaches pure
PE occupancy.

Attention tiles are emitted at the exact causal triangle (ragged diagonal
chunks: for query tile j, diagonal key chunk pi covers only queries
128*pi..512, via ragged matmul + Exp + ragged PV accumulation) instead of
full-width masked diagonals - 15% less score/PV/Exp work.

PSUM budget (8 banks): v/qk GEMM accumulators share one 4-buf tag (v uses
4 simultaneously, qk rotates 8 allocations through them), scores use a
3-deep single-chunk [128,512] pipeline, and aps/lps/rps share one bank
(their lifetimes are strictly sequential within a tile's tail).

Device notes:
- All GEMM operands float16 (1 PE cycle/row; fp32 PSUM accumulation).
- q/k/v stay SBUF-resident between the QKV phase and the attention phase.
- All DRAM operands are host-preblocked so DMA lands as long contiguous
  per-partition runs (the HWDGE is descriptor-rate-bound on transposed
  access patterns).
- Softmax is max-free: Exp on ScalarE straight out of PSUM; denominators
  accumulated into a single [128,TQ] SBUF accumulator on VectorE and
  reduced with one ones-vector matmul per query tile; the reciprocal
  broadcast is one ones-row matmul per tile. The tail stages are emitted
  one interleave-slot apart so the in-order PE never waits on VectorE.
"""

import numpy as np

import concourse.bass as bass
import concourse.bass_isa as bass_isa
import concourse.tile as tile
from concourse import bacc, mybir
from concourse.bass_utils import run_bass_kernel_spmd

HIDDEN = 4096
N_HEADS = 32
HEAD_DIM = 128
B, S = 2, 2048
T = B * S
N_CORES = 8
HPC = N_HEADS // N_CORES  # 4 head indices per core (both batches)
ROPE_BASE = 10000.0

FP = mybir.dt.float16
F32 = mybir.dt.float32

M_B = 512                # token block in QKV GEMM
NBLK = T // M_B          # 8 blocks
KC = HIDDEN // 128       # 32 contraction chunks
TQ = 512                 # query tile in attention
SCHUNKS = S // 128       # 16 key chunks per batch


def build_nc1():
    """QKV projection + RoPE + causal attention, fused. Output: attention
    results in 8-shard layout attnT[s, :, :] = shard for (batch s//4,
    tokblock s%4), rows = this core's 4 heads x 128 dims, cols = 512
    tokens (fp16)."""
    nc = bacc.Bacc(num_devices=N_CORES, trn_type="TRN2")

    xT = nc.declare_dram_parameter("xT", [128, NBLK, KC, M_B], FP,
                                   isOutput=False)
    wqk = nc.declare_dram_parameter("wqk", [8, 128, KC, 128], FP, isOutput=False)
    wv = nc.declare_dram_parameter("wv", [128, KC, 512], FP, isOutput=False)
    cosq = nc.declare_dram_parameter("cosq", [128, T], FP, isOutput=False)
    sinq = nc.declare_dram_parameter("sinq", [128, T], FP, isOutput=False)
    cosk = nc.declare_dram_parameter("cosk", [128, T], FP, isOutput=False)
    sink = nc.declare_dram_parameter("sink", [128, T], FP, isOutput=False)
    trimask = nc.declare_dram_parameter("trimask", [128, 128], FP,
                                        isOutput=False)
    onesc = nc.declare_dram_parameter("onesc", [128, 1], FP, isOutput=False)
    onesr = nc.declare_dram_parameter("onesr", [1, 128], FP, isOutput=False)
    attnT = nc.declare_dram_parameter("attnT", [8, 512, 512], FP, isOutput=True)

    with nc.allow_low_precision(reason="float16 carries the data; fp32 PSUM"), \
         tile.TileContext(nc) as tc:
        with tc.tile_pool(name="res", bufs=1) as rpool_res, \
             tc.tile_pool(name="xblk", bufs=1) as xpool, \
             tc.tile_pool(name="wq", bufs=1) as wqpool, \
             tc.tile_pool(name="wvp", bufs=1) as wvpool, \
             tc.tile_pool(name="rope", bufs=1) as ropool, \
             tc.tile_pool(name="tmpp", bufs=1) as tpool, \
             tc.tile_pool(name="probs", bufs=1) as ppool, \
             tc.tile_pool(name="accp", bufs=1) as apool, \
             tc.tile_pool(name="yt", bufs=3) as ypool, \
             tc.tile_pool(name="ps", bufs=1, space="PSUM") as ps:
            # q/k feature-major [head_dim, 4 heads, T]; v token-major
            # [128 tok, 32 chunks, 4 heads x 128 dims]. SBUF-resident.
            q_res = rpool_res.tile([128, HPC, T], FP, name="q_res")
            k_res = rpool_res.tile([128, HPC, T], FP, name="k_res")
            v_res = rpool_res.tile([128, T // 128, 512], FP, name="v_res")
            ones_col = rpool_res.tile([128, 1], FP, name="ones_col")
            ones_row = rpool_res.tile([1, 128], FP, name="ones_row")
            msk = rpool_res.tile([128, 128], FP, name="msk")

            # ---- attention package for one (batch, query-tile): a list of
            # emission steps (closures) pumped between QKV emission slots ----
            def tile_steps(bb, jb, h):
                base = bb * S
                q0 = base + jb * TQ
                nchunks = 4 * jb + 4
                st = {"aps": None, "acc": None, "prs": {}}

                def lo_of(c):
                    pi = c - 4 * jb
                    return 128 * pi if pi >= 0 else 0

                def sc(c):
                    lo = lo_of(c)
                    sps = ps.tile([128, TQ], F32, tag="sps", bufs=3,
                                  name="sps")
                    nc.tensor.matmul(
                        sps[:, lo:TQ],
                        k_res[:, h, base + c * 128: base + (c + 1) * 128],
                        q_res[:, h, q0 + lo: q0 + TQ],
                        start=True, stop=True)
                    pr = ppool.tile([128, TQ], FP, tag="pr", bufs=4,
                                    name="pr")
                    nc.scalar.activation(pr[:, lo:TQ], sps[:, lo:TQ],
                                         mybir.ActivationFunctionType.Exp)
                    if c - 4 * jb >= 0:
                        # triangular block on the exact diagonal
                        nc.vector.tensor_mul(pr[:, lo:lo + 128],
                                             pr[:, lo:lo + 128], msk[:])
                    if c == 0:
                        acc = apool.tile([128, TQ], FP, tag="acc", bufs=2)
                        st["acc"] = acc
                        nc.vector.tensor_copy(acc[:], pr[:])
                    else:
                        acc = st["acc"]
                        nc.vector.tensor_add(acc[:, lo:TQ], acc[:, lo:TQ],
                                             pr[:, lo:TQ])
                    st["prs"][c] = pr

                def pv(c):
                    lo = lo_of(c)
                    pr = st["prs"].pop(c)
                    if st["aps"] is None:
                        st["aps"] = ps.tile([128, TQ], F32, tag="aps",
                                            bufs=1, name="aps")
                    nc.tensor.matmul(
                        st["aps"][:, lo:TQ],
                        v_res[:, bb * SCHUNKS + c, h * 128:(h + 1) * 128],
                        pr[:, lo:TQ],
                        start=(c == 0), stop=(c == nchunks - 1))

                # software pipeline: scores run 2 chunks ahead of PV so the
                # in-order PE has independent work while Exp runs on ACT.
                # Steps are (fn, solo): solo steps end their pump batch, so
                # the tail matmuls (lps waits on the VectorE accumulator,
                # rps on the ScalarE r-copy) always get a full QKV emission
                # slot of independent PE work between them.
                steps = []
                steps.append((lambda: sc(0), False))
                if nchunks > 1:
                    steps.append((lambda: sc(1), False))
                for c in range(2, nchunks):
                    steps.append((lambda c=c: (sc(c), pv(c - 2)), False))
                def fin0():
                    if nchunks > 1:
                        pv(nchunks - 2)
                    pv(nchunks - 1)
                    aps = st["aps"]
                    araw = ypool.tile([128, TQ], FP, tag="araw", bufs=3)
                    # alternate eviction between ScalarE and VectorE
                    if (jb + h) % 2 == 0:
                        nc.scalar.copy(araw[:], aps[:])
                    else:
                        nc.vector.tensor_copy(araw[:], aps[:])
                    st["araw"] = araw
                steps.append((fin0, True))
                def fin1():
                    lps = ps.tile([128, TQ], F32, tag="aps", bufs=1,
                                  name="lps")
                    nc.tensor.matmul(lps[0:1, :], ones_col[:], st["acc"][:],
                                     start=True, stop=True)
                    r = ypool.tile([1, TQ], FP, tag="r", bufs=2)
                    nc.scalar.copy(r[:], lps[0:1, :])
                    st["r"] = r
                steps.append((fin1, True))
                def fin2():
                    rps = ps.tile([128, TQ], F32, tag="aps", bufs=1,
                                  name="rps")
                    nc.tensor.matmul(rps[:], ones_row[:], st["r"][:],
                                     start=True, stop=True)
                    rrep = ypool.tile([128, TQ], F32, tag="rrep", bufs=1)
                    nc.vector.reciprocal_approx_fast(rrep[:], rps[:])
                    yt = ypool.tile([128, TQ], FP, tag="yt", bufs=2)
                    nc.vector.tensor_mul(yt[:], st["araw"][:], rrep[:])
                    # ACT-queue HWDGE: keeps the sync queue pure loads, so a
                    # not-yet-ready yt never head-of-line blocks weight DMAs
                    nc.scalar.dma_start(
                        attnT[bb * 4 + jb, h * 128:(h + 1) * 128, :], yt[:])
                steps.append((fin2, True))
                return steps

            pending = []

            def pump(slots_left):
                # drain pending package steps evenly across remaining slots
                n = -(-len(pending) // slots_left) if slots_left > 0 else \
                    len(pending)
                for _ in range(min(n, len(pending))):
                    fn, solo = pending.pop(0)
                    fn()
                    if solo:
                        break

            # ---------------- fused QKV + attention loop ----------------
            for blk in range(NBLK):
                t0 = blk * M_B
                xb = xpool.tile([128, KC, M_B], FP, tag="xb", bufs=2)
                # chunked load: first v matmul starts after ~0.5MB lands
                for g in range(8):
                    nc.sync.dma_start(
                        xb[:, g * 4:(g + 1) * 4, :],
                        xT[:, blk, g * 4:(g + 1) * 4, :])
                if blk == 0:
                    nc.sync.dma_start(ones_col[:], onesc[:])
                    nc.sync.dma_start(ones_row[:], onesr[:])
                    nc.sync.dma_start(msk[:], trimask[:])
                # rope tables for this block
                cq = ropool.tile([128, M_B], FP, tag="cq", bufs=2)
                sq = ropool.tile([128, M_B], FP, tag="sq", bufs=2)
                ck = ropool.tile([128, M_B], FP, tag="ck", bufs=2)
                sk = ropool.tile([128, M_B], FP, tag="sk", bufs=2)
                nc.sync.dma_start(cq[:], cosq[:, t0:t0 + M_B])
                nc.sync.dma_start(sq[:], sinq[:, t0:t0 + M_B])
                nc.sync.dma_start(ck[:], cosk[:, t0:t0 + M_B])
                nc.sync.dma_start(sk[:], sink[:, t0:t0 + M_B])

                nslots = 16

                # v: token-major [tok 128, feat 512], accumulate over kc;
                # the 4 token-subtile PSUM accumulators live for the whole
                # kc loop and share the "vqk" tag with the qk PSUM tiles
                vps = []
                for tt in range(4):
                    vps.append(ps.tile([128, 512], F32, tag="vqk",
                                       bufs=4, name=f"vps{tt}"))
                for g in range(KC // 2):
                    wvt = wvpool.tile([128, 2, 512], FP, tag="wv", bufs=3)
                    nc.sync.dma_start(wvt[:], wv[:, g * 2:(g + 1) * 2, :])
                    for i in range(2):
                        kc = g * 2 + i
                        for tt in range(4):
                            nc.tensor.matmul(
                                vps[tt][:],
                                xb[:, kc, tt * 128:(tt + 1) * 128],
                                wvt[:, i, :],
                                start=(kc == 0), stop=(kc == KC - 1))
                    if g % 2 == 1:
                        pump(nslots)
                        nslots -= 1
                for tt in range(4):
                    nc.vector.tensor_copy(v_res[:, blk * 4 + tt, :],
                                          vps[tt][:])

                # q, k: feature-major [head_dim 128, tok 512] + fused RoPE
                for ft in range(8):
                    qp = ps.tile([128, M_B], F32, tag="vqk", bufs=4,
                                 name="qkps")
                    wt = wqpool.tile([128, KC, 128], FP, tag="wqk", bufs=2)
                    nc.sync.dma_start(wt[:], wqk[ft])
                    for kc in range(KC):
                        nc.tensor.matmul(qp[:], wt[:, kc, :], xb[:, kc, :],
                                         start=(kc == 0), stop=(kc == KC - 1))
                    is_q = ft < 4
                    h = ft % 4
                    cos2, sin2 = (cq, sq) if is_q else (ck, sk)
                    dst = (q_res if is_q else k_res)[:, h, t0:t0 + M_B]
                    tmp = tpool.tile([128, M_B], FP, tag="ropetmp", bufs=2)
                    nc.vector.tensor_mul(dst, qp[:], cos2[:])
                    nc.vector.tensor_mul(tmp[0:64, :], qp[64:128, :],
                                         sin2[0:64, :])
                    nc.vector.tensor_mul(tmp[64:128, :], qp[0:64, :],
                                         sin2[64:128, :])
                    nc.vector.tensor_sub(dst[0:64], dst[0:64], tmp[0:64, :])
                    nc.vector.tensor_add(dst[64:128], dst[64:128],
                                         tmp[64:128, :])
                    pump(nslots)
                    nslots -= 1

                # enqueue this block's attention package; it is pumped into
                # the next block's QKV emission slots (the final block's
                # package drains bare after the loop)
                bb, jb = divmod(blk, 4)
                for h in range(HPC):
                    pending.extend(tile_steps(bb, jb, h))

            while pending:
                pending.pop(0)[0]()

    nc.finalize()
    return nc


def build_nc2():
    """o_proj: out[of, tok] = sum_f WoT[f, of] * yT[f, tok] for this core's
    (batch, token-block) slice, with the full o_proj weight."""
    nc = bacc.Bacc(num_devices=N_CORES, trn_type="TRN2")
    yT = nc.declare_dram_parameter("yT", [128, KC, 512], FP, isOutput=False)
    wo = nc.declare_dram_parameter("wo", [32, 128, KC, 128], FP, isOutput=False)
    out = nc.declare_dram_parameter("out", [HIDDEN, 512], FP, isOutput=True)

    with nc.allow_low_precision(reason="float16 output; fp32 PSUM"), \
         tile.TileContext(nc) as tc:
        with tc.tile_pool(name="yblk", bufs=1) as ybpool, \
             tc.tile_pool(name="wop", bufs=4) as wopool, \
             tc.tile_pool(name="oev", bufs=3) as oepool, \
             tc.tile_pool(name="psD", bufs=1, space="PSUM") as psD:
            # first weight tile before the activation block: the of=0 matmuls
            # gate on wt0 + the first ysb chunk only
            wts = []
            for pre in range(2):
                wt = wopool.tile([128, KC, 128], FP, tag="wo", bufs=3)
                nc.sync.dma_start(wt[:], wo[pre])
                wts.append(wt)
            ysb = ybpool.tile([128, KC, 512], FP)
            for kq in range(8):
                nc.sync.dma_start(
                    ysb[:, kq * 4:(kq + 1) * 4, :],
                    yT[:, kq * 4:(kq + 1) * 4, :])
            for of in range(32):
                ops = psD.tile([128, 512], F32, tag="ops", bufs=4)
                if of < 2:
                    wt = wts[of]
                else:
                    wt = wopool.tile([128, KC, 128], FP, tag="wo", bufs=3)
                    nc.sync.dma_start(wt[:], wo[of])
                for kc in range(KC):
                    nc.tensor.matmul(ops[:], wt[:, kc, :], ysb[:, kc, :],
                                     start=(kc == 0), stop=(kc == KC - 1))
                osb = oepool.tile([128, 512], FP, tag="oev")
                nc.vector.tensor_copy(osb[:], ops[:])
                nc.sync.dma_start(out[of * 128:(of + 1) * 128, :], osb[:])

    nc.finalize()
    return nc


_NC1 = None
_NC2 = None


def get_ncs():
    global _NC1, _NC2
    if _NC1 is None:
        _NC1 = build_nc1()
        _NC2 = build_nc2()
    return _NC1, _NC2


def _rope_tables(positions):
    """positions [B, S] int -> packed cos/sin tables [128, T] f32 in token
    order (b*S + t); rows [0:64] and [64:128] hold the same 64 freqs."""
    inv_freq = 1.0 / (ROPE_BASE ** (np.arange(0, HEAD_DIM, 2, dtype=np.float64)
                                    / HEAD_DIM))
    freqs = np.asarray(positions).reshape(T).astype(np.float64)[:, None] * inv_freq
    cos = np.cos(freqs).T.astype(np.float32)  # [64, T]
    sin = np.sin(freqs).T.astype(np.float32)
    cos2 = np.concatenate([cos, cos], axis=0)  # [128, T]
    sin2 = np.concatenate([sin, sin], axis=0)
    scale = np.float32(HEAD_DIM ** -0.5)
    return ((cos2 * scale).astype(np.float16), (sin2 * scale).astype(np.float16),
            cos2.astype(np.float16), sin2.astype(np.float16))


def prepare_inputs1(hidden_states, positions, W_pack):
    x = np.ascontiguousarray(np.asarray(hidden_states, dtype=np.float32)
                             .reshape(T, HIDDEN))
    # [p, blk, kc, t]: per-partition-contiguous blocks (cheap DMA descriptors)
    xT_blocks = np.ascontiguousarray(
        x.T.astype(np.float16).reshape(KC, 128, NBLK, M_B)
        .transpose(1, 2, 0, 3))

    cosq, sinq, cosk, sink = _rope_tables(positions)

    tri = (np.arange(128)[:, None] <= np.arange(128)[None, :]) \
        .astype(np.float16)

    in_maps = []
    for c in range(N_CORES):
        hs = [HPC * c + i for i in range(HPC)]
        wqk_blocks = np.empty((8, 128, KC, 128), dtype=np.float16)
        for ft in range(8):
            off = 0 if ft < 4 else HIDDEN
            h = hs[ft % 4]
            wsl = W_pack[off + h * 128: off + (h + 1) * 128, :]  # [128, 4096]
            wqk_blocks[ft] = wsl.reshape(128, KC, 128).transpose(2, 1, 0)
        wv_sl = np.concatenate(
            [W_pack[2 * HIDDEN + h * 128: 2 * HIDDEN + (h + 1) * 128, :]
             for h in hs], axis=0)  # [512, 4096]
        wv_blocks = np.ascontiguousarray(
            wv_sl.astype(np.float16).reshape(512, KC, 128).transpose(2, 1, 0))
        in_maps.append({
            "xT": xT_blocks,
            "wqk": np.ascontiguousarray(wqk_blocks),
            "wv": wv_blocks,
            "cosq": cosq, "sinq": sinq, "cosk": cosk, "sink": sink,
            "trimask": tri,
            "onesc": np.ones((128, 1), dtype=np.float16),
            "onesr": np.ones((1, 128), dtype=np.float16),
        })
    return in_maps


def prepare_inputs2(res1, W_o):
    """Host-side shard permutation (the "AllToAll"): pure gather, no math."""
    wo_blocks = np.ascontiguousarray(
        np.ascontiguousarray(np.asarray(W_o, dtype=np.float32).T
                             .astype(np.float16))
        .reshape(KC, 128, 32, 128).transpose(2, 1, 0, 3))
    in_maps2 = []
    for j in range(N_CORES):
        yT = np.concatenate([res1.results[c]["attnT"][j] for c in range(N_CORES)],
                            axis=0)  # [4096, 512] feature-major, head order
        yTp = np.ascontiguousarray(
            yT.reshape(KC, 128, 512).transpose(1, 0, 2))  # [p, kc, t]
        in_maps2.append({"yT": yTp, "wo": wo_blocks})
    return in_maps2


def assemble(res2):
    out = np.empty((B, S, HIDDEN), dtype=np.float32)
    for c in range(N_CORES):
        bb, j = c // 4, c % 4
        out[bb, j * 512:(j + 1) * 512, :] = \
            res2.results[c]["out"].T.astype(np.float32)
    return out


def run(hidden_states, positions, W_pack, W_o, trace=False):
    nc1, nc2 = get_ncs()
    in_maps1 = prepare_inputs1(hidden_states, positions,
                               np.asarray(W_pack, dtype=np.float32))
    res1 = run_bass_kernel_spmd(nc1, in_maps1, list(range(N_CORES)),
                                trace=trace)
    in_maps2 = prepare_inputs2(res1, W_o)
    res2 = run_bass_kernel_spmd(nc2, in_maps2, list(range(N_CORES)),
                                trace=trace)
    return assemble(res2), res1, res2


def kernel(hidden_states, positions, W_pack, W_o):
    out, _, _ = run(hidden_states, positions, W_pack, W_o)
    return out


# revision 14
# speedup vs baseline: 1.0204x; 1.0116x over previous
"""Causal self-attention block (QKV proj + RoPE + causal attention + o_proj)
on 8 Trainium2 NeuronCores.

Sharding: tensor-parallel over heads for QKV+attention (core c owns head
indices 4c..4c+3 of BOTH batch elements). The attention outputs are written
in an 8-shard layout; the host permutes shards (a pure gather, the moral
equivalent of an AllToAll - on-device collectives fail to load under this
runtime) so that launch 2 runs o_proj token-sharded: core c handles
(batch c//4, token block c%4) with the full o_proj weight. The final host
step is a pure concatenation of the 8 disjoint output slices.

Launch-1 structure (the big change vs the phase-serial version): the
attention work for token block g is INTERLEAVED into token block g+1's QKV
GEMM emission. The QKV phase is pure PE work (~660us) with idle ScalarE/
VectorE; the attention phase alone is jointly PE+ScalarE+VectorE bound
(~205us span over ~160us of PE). Fused, the attention Exp/accumulate work
hides completely under the QKV matmul stream and launch 1 approaches pure
PE occupancy.

Attention tiles are emitted at the exact causal triangle (ragged diagonal
chunks: for query tile j, diagonal key chunk pi covers only queries
128*pi..512, via ragged matmul + Exp + ragged PV accumulation) instead of
full-width masked diagonals - 15% less score/PV/Exp work.

PSUM budget (8 banks): v/qk GEMM accumulators share one 4-buf tag (v uses
4 simultaneously, qk rotates 8 allocations through them), scores use a
3-deep single-chunk [128,512] pipeline, and aps/lps/rps share one bank
(their lifetimes are strictly sequential within a tile's tail).

Device notes:
- All GEMM operands float16 (1 PE cycle/row; fp32 PSUM accumulation).
- q/k/v stay SBUF-resident between the QKV phase and the attention phase.
- All DRAM operands are host-preblocked so DMA lands as long contiguous
  per-partition runs (the HWDGE is descriptor-rate-bound on transposed
  access patterns).
- Softmax is max-free: Exp on ScalarE straight out of PSUM; denominators
  accumulated into a single [128,TQ] SBUF accumulator on VectorE and
  reduced with one ones-vector matmul per query tile; the reciprocal
  broadcast is one ones-row matmul per tile. The tail stages are emitted
  one interleave-slot apart so the in-order PE never waits on VectorE.
"""

import numpy as np

import concourse.bass as bass
import concourse.bass_isa as bass_isa
import concourse.tile as tile
from concourse import bacc, mybir
from concourse.bass_utils import run_bass_kernel_spmd

HIDDEN = 4096
N_HEADS = 32
HEAD_DIM = 128
B, S = 2, 2048
T = B * S
N_CORES = 8
HPC = N_HEADS // N_CORES  # 4 head indices per core (both batches)
ROPE_BASE = 10000.0

FP = mybir.dt.float16
F32 = mybir.dt.float32

M_B = 512                # token block in QKV GEMM
NBLK = T // M_B          # 8 blocks
KC = HIDDEN // 128       # 32 contraction chunks
TQ = 512                 # query tile in attention
SCHUNKS = S // 128       # 16 key chunks per batch


def build_nc1():
    """QKV projection + RoPE + causal attention, fused. Output: attention
    results in 8-shard layout attnT[s, :, :] = shard for (batch s//4,
    tokblock s%4), rows = this core's 4 heads x 128 dims, cols = 512
    tokens (fp16)."""
    nc = bacc.Bacc(num_devices=N_CORES, trn_type="TRN2")

    xT = nc.declare_dram_parameter("xT", [128, NBLK, KC, M_B], FP,
                                   isOutput=False)
    wqk = nc.declare_dram_parameter("wqk", [8, 128, KC, 128], FP, isOutput=False)
    wv = nc.declare_dram_parameter("wv", [128, KC, 512], FP, isOutput=False)
    cosq = nc.declare_dram_parameter("cosq", [128, T], FP, isOutput=False)
    sinq = nc.declare_dram_parameter("sinq", [128, T], FP, isOutput=False)
    cosk = nc.declare_dram_parameter("cosk", [128, T], FP, isOutput=False)
    sink = nc.declare_dram_parameter("sink", [128, T], FP, isOutput=False)
    trimask = nc.declare_dram_parameter("trimask", [128, 128], FP,
                                        isOutput=False)
    onesc = nc.declare_dram_parameter("onesc", [128, 1], FP, isOutput=False)
    onesr = nc.declare_dram_parameter("onesr", [1, 128], FP, isOutput=False)
    attnT = nc.declare_dram_parameter("attnT", [8, 512, 512], FP, isOutput=True)

    with nc.allow_low_precision(reason="float16 carries the data; fp32 PSUM"), \
         tile.TileContext(nc) as tc:
        with tc.tile_pool(name="res", bufs=1) as rpool_res, \
             tc.tile_pool(name="xblk", bufs=1) as xpool, \
             tc.tile_pool(name="wq", bufs=1) as wqpool, \
             tc.tile_pool(name="wvp", bufs=1) as wvpool, \
             tc.tile_pool(name="rope", bufs=1) as ropool, \
             tc.tile_pool(name="tmpp", bufs=1) as tpool, \
             tc.tile_pool(name="probs", bufs=1) as ppool, \
             tc.tile_pool(name="accp", bufs=1) as apool, \
             tc.tile_pool(name="yt", bufs=3) as ypool, \
             tc.tile_pool(name="ps", bufs=1, space="PSUM") as ps:
            # q/k feature-major [head_dim, 4 heads, T]; v token-major
            # [128 tok, 32 chunks, 4 heads x 128 dims]. SBUF-resident.
            q_res = rpool_res.tile([128, HPC, T], FP, name="q_res")
            k_res = rpool_res.tile([128, HPC, T], FP, name="k_res")
            v_res = rpool_res.tile([128, T // 128, 512], FP, name="v_res")
            ones_col = rpool_res.tile([128, 1], FP, name="ones_col")
            ones_row = rpool_res.tile([1, 128], FP, name="ones_row")
            msk = rpool_res.tile([128, 128], FP, name="msk")

            # ---- attention package for one (batch, query-tile): a list of
            # emission steps (closures) pumped between QKV emission slots ----
            def tile_steps(bb, jb, h):
                base = bb * S
                q0 = base + jb * TQ
                nchunks = 4 * jb + 4
                st = {"aps": None, "acc": None, "prs": {}}

                def lo_of(c):
                    pi = c - 4 * jb
                    return 128 * pi if pi >= 0 else 0

                def sc(c):
                    lo = lo_of(c)
                    sps = ps.tile([128, TQ], F32, tag="sps", bufs=3,
                                  name="sps")
                    nc.tensor.matmul(
                        sps[:, lo:TQ],
                        k_res[:, h, base + c * 128: base + (c + 1) * 128],
                        q_res[:, h, q0 + lo: q0 + TQ],
                        start=True, stop=True)
                    pr = ppool.tile([128, TQ], FP, tag="pr", bufs=4,
                                    name="pr")
                    nc.scalar.activation(pr[:, lo:TQ], sps[:, lo:TQ],
                                         mybir.ActivationFunctionType.Exp)
                    if c - 4 * jb >= 0:
                        # triangular block on the exact diagonal
                        nc.vector.tensor_mul(pr[:, lo:lo + 128],
                                             pr[:, lo:lo + 128], msk[:])
                    if c == 0:
                        acc = apool.tile([128, TQ], FP, tag="acc", bufs=2)
                        st["acc"] = acc
                        nc.vector.tensor_copy(acc[:], pr[:])
                    else:
                        acc = st["acc"]
                        nc.vector.tensor_add(acc[:, lo:TQ], acc[:, lo:TQ],
                                             pr[:, lo:TQ])
                    st["prs"][c] = pr

                def pv(c):
                    lo = lo_of(c)
                    pr = st["prs"].pop(c)
                    if st["aps"] is None:
                        st["aps"] = ps.tile([128, TQ], F32, tag="aps",
                                            bufs=1, name="aps")
                    nc.tensor.matmul(
                        st["aps"][:, lo:TQ],
                        v_res[:, bb * SCHUNKS + c, h * 128:(h + 1) * 128],
                        pr[:, lo:TQ],
                        start=(c == 0), stop=(c == nchunks - 1))

                # software pipeline: scores run 2 chunks ahead of PV so the
                # in-order PE has independent work while Exp runs on ACT.
                # Steps are (fn, solo): solo steps end their pump batch, so
                # the tail matmuls (lps waits on the VectorE accumulator,
                # rps on the ScalarE r-copy) always get a full QKV emission
                # slot of independent PE work between them.
                steps = []
                steps.append((lambda: sc(0), False))
                if nchunks > 1:
                    steps.append((lambda: sc(1), False))
                for c in range(2, nchunks):
                    steps.append((lambda c=c: (sc(c), pv(c - 2)), False))
                def fin0():
                    if nchunks > 1:
                        pv(nchunks - 2)
                    pv(nchunks - 1)
                    aps = st["aps"]
                    araw = ypool.tile([128, TQ], FP, tag="araw", bufs=3)
                    # alternate eviction between ScalarE and VectorE
                    if (jb + h) % 2 == 0:
                        nc.scalar.copy(araw[:], aps[:])
                    else:
                        nc.vector.tensor_copy(araw[:], aps[:])
                    st["araw"] = araw
                steps.append((fin0, True))
                def fin1():
                    lps = ps.tile([128, TQ], F32, tag="aps", bufs=1,
                                  name="lps")
                    nc.tensor.matmul(lps[0:1, :], ones_col[:], st["acc"][:],
                                     start=True, stop=True)
                    r = ypool.tile([1, TQ], FP, tag="r", bufs=2)
                    nc.scalar.copy(r[:], lps[0:1, :])
                    st["r"] = r
                steps.append((fin1, True))
                def fin2():
                    rps = ps.tile([128, TQ], F32, tag="aps", bufs=1,
                                  name="rps")
                    nc.tensor.matmul(rps[:], ones_row[:], st["r"][:],
                                     start=True, stop=True)
                    rrep = ypool.tile([128, TQ], F32, tag="rrep", bufs=1)
                    nc.vector.reciprocal_approx_fast(rrep[:], rps[:])
                    yt = ypool.tile([128, TQ], FP, tag="yt", bufs=2)
                    nc.vector.tensor_mul(yt[:], st["araw"][:], rrep[:])
                    # ACT-queue HWDGE: keeps the sync queue pure loads, so a
                    # not-yet-ready yt never head-of-line blocks weight DMAs
                    nc.scalar.dma_start(
                        attnT[bb * 4 + jb, h * 128:(h + 1) * 128, :], yt[:])
                steps.append((fin2, True))
                return steps

            pending = []

            def pump(slots_left):
                # drain pending package steps evenly across remaining slots
                n = -(-len(pending) // slots_left) if slots_left > 0 else \
                    len(pending)
                for _ in range(min(n, len(pending))):
                    fn, solo = pending.pop(0)
                    fn()
                    if solo:
                        break

            # ---------------- fused QKV + attention loop ----------------
            for blk in range(NBLK):
                t0 = blk * M_B
                last = blk == NBLK - 1
                bb, jb = divmod(blk, 4)
                xb = xpool.tile([128, KC, M_B], FP, tag="xb", bufs=2)

                def xchunk(c2):
                    nc.sync.dma_start(
                        xb[:, c2 * 4:(c2 + 1) * 4, :],
                        xT[:, blk, c2 * 4:(c2 + 1) * 4, :])
                # DMA triggers cost ~600ns of sync-engine time each and only
                # 8 completion-sem lanes exist, so emission ORDER is arrival
                # order: for block 0 the first v matmul gates on xb chunk 0
                # + wvt group 0, so those must be the first two triggers
                # (everything else is interleaved/deferred); later blocks
                # have a full block of DMA lead time.
                if blk == 0:
                    xchunk(0)
                else:
                    for c2 in range(8):
                        xchunk(c2)

                nslots = 16

                # v: token-major [tok 128, feat 512], accumulate over kc;
                # the 4 token-subtile PSUM accumulators live for the whole
                # kc loop and share the "vqk" tag with the qk PSUM tiles
                vps = []
                for tt in range(4):
                    vps.append(ps.tile([128, 512], F32, tag="vqk",
                                       bufs=4, name=f"vps{tt}"))
                for g in range(KC // 2):
                    wvt = wvpool.tile([128, 2, 512], FP, tag="wv", bufs=3)
                    nc.sync.dma_start(wvt[:], wv[:, g * 2:(g + 1) * 2, :])
                    if blk == 0 and g == 0:
                        xchunk(1)
                    if blk == 0 and g % 2 == 1 and 2 <= (g + 3) // 2 <= 7:
                        xchunk((g + 3) // 2)
                    for i in range(2):
                        kc = g * 2 + i
                        for tt in range(4):
                            nc.tensor.matmul(
                                vps[tt][:],
                                xb[:, kc, tt * 128:(tt + 1) * 128],
                                wvt[:, i, :],
                                start=(kc == 0), stop=(kc == KC - 1))
                    if g % 2 == 1:
                        pump(nslots)
                        nslots -= 1
                for tt in range(4):
                    nc.vector.tensor_copy(v_res[:, blk * 4 + tt, :],
                                          vps[tt][:])

                # aux + rope tables: first needed in the ft section / by the
                # first pumped package, so they trail the v-section loads
                if blk == 0:
                    nc.sync.dma_start(ones_col[:], onesc[:])
                    nc.sync.dma_start(ones_row[:], onesr[:])
                    nc.sync.dma_start(msk[:], trimask[:])
                cq = ropool.tile([128, M_B], FP, tag="cq", bufs=2)
                sq = ropool.tile([128, M_B], FP, tag="sq", bufs=2)
                ck = ropool.tile([128, M_B], FP, tag="ck", bufs=2)
                sk = ropool.tile([128, M_B], FP, tag="sk", bufs=2)
                nc.sync.dma_start(cq[:], cosq[:, t0:t0 + M_B])
                nc.sync.dma_start(sq[:], sinq[:, t0:t0 + M_B])
                nc.sync.dma_start(ck[:], cosk[:, t0:t0 + M_B])
                nc.sync.dma_start(sk[:], sink[:, t0:t0 + M_B])

                # q, k: feature-major [head_dim 128, tok 512] + fused RoPE.
                # Final block: interleave [q_h, k_h] pairs and enqueue head
                # h's attention package as soon as its pair is emitted, so
                # most of the last packages still interleave with QKV work
                # instead of draining bare at the end of the launch.
                ft_order = [0, 4, 1, 5, 2, 6, 3, 7] if last else range(8)
                for idx, ft in enumerate(ft_order):
                    qp = ps.tile([128, M_B], F32, tag="vqk", bufs=4,
                                 name="qkps")
                    wt = wqpool.tile([128, KC, 128], FP, tag="wqk", bufs=2)
                    nc.sync.dma_start(wt[:], wqk[ft])
                    for kc in range(KC):
                        nc.tensor.matmul(qp[:], wt[:, kc, :], xb[:, kc, :],
                                         start=(kc == 0), stop=(kc == KC - 1))
                    is_q = ft < 4
                    h = ft % 4
                    cos2, sin2 = (cq, sq) if is_q else (ck, sk)
                    dst = (q_res if is_q else k_res)[:, h, t0:t0 + M_B]
                    tmp = tpool.tile([128, M_B], FP, tag="ropetmp", bufs=2)
                    nc.vector.tensor_mul(dst, qp[:], cos2[:])
                    nc.vector.tensor_mul(tmp[0:64, :], qp[64:128, :],
                                         sin2[0:64, :])
                    nc.vector.tensor_mul(tmp[64:128, :], qp[0:64, :],
                                         sin2[64:128, :])
                    nc.vector.tensor_sub(dst[0:64], dst[0:64], tmp[0:64, :])
                    nc.vector.tensor_add(dst[64:128], dst[64:128],
                                         tmp[64:128, :])
                    if last and idx % 2 == 1:
                        pending.extend(tile_steps(bb, jb, idx // 2))
                    pump(nslots)
                    nslots -= 1

                # enqueue this block's attention package; it is pumped into
                # the next block's QKV emission slots
                if not last:
                    for h in range(HPC):
                        pending.extend(tile_steps(bb, jb, h))

            while pending:
                pending.pop(0)[0]()

    nc.finalize()
    return nc


def build_nc2():
    """o_proj: out[of, tok] = sum_f WoT[f, of] * yT[f, tok] for this core's
    (batch, token-block) slice, with the full o_proj weight."""
    nc = bacc.Bacc(num_devices=N_CORES, trn_type="TRN2")
    yT = nc.declare_dram_parameter("yT", [128, KC, 512], FP, isOutput=False)
    wo = nc.declare_dram_parameter("wo", [32, 128, KC, 128], FP, isOutput=False)
    out = nc.declare_dram_parameter("out", [HIDDEN, 512], FP, isOutput=True)

    with nc.allow_low_precision(reason="float16 output; fp32 PSUM"), \
         tile.TileContext(nc) as tc:
        with tc.tile_pool(name="yblk", bufs=1) as ybpool, \
             tc.tile_pool(name="wop", bufs=4) as wopool, \
             tc.tile_pool(name="oev", bufs=3) as oepool, \
             tc.tile_pool(name="psD", bufs=1, space="PSUM") as psD:
            # first weight tile before the activation block: the of=0 matmuls
            # gate on wt0 + the first ysb chunk only
            wts = []
            for pre in range(2):
                wt = wopool.tile([128, KC, 128], FP, tag="wo", bufs=3)
                nc.sync.dma_start(wt[:], wo[pre])
                wts.append(wt)
            ysb = ybpool.tile([128, KC, 512], FP)
            for kq in range(8):
                nc.sync.dma_start(
                    ysb[:, kq * 4:(kq + 1) * 4, :],
                    yT[:, kq * 4:(kq + 1) * 4, :])
            for of in range(32):
                ops = psD.tile([128, 512], F32, tag="ops", bufs=4)
                if of < 2:
                    wt = wts[of]
                else:
                    wt = wopool.tile([128, KC, 128], FP, tag="wo", bufs=3)
                    nc.sync.dma_start(wt[:], wo[of])
                for kc in range(KC):
                    nc.tensor.matmul(ops[:], wt[:, kc, :], ysb[:, kc, :],
                                     start=(kc == 0), stop=(kc == KC - 1))
                osb = oepool.tile([128, 512], FP, tag="oev")
                nc.vector.tensor_copy(osb[:], ops[:])
                nc.sync.dma_start(out[of * 128:(of + 1) * 128, :], osb[:])

    nc.finalize()
    return nc


_NC1 = None
_NC2 = None


def get_ncs():
    global _NC1, _NC2
    if _NC1 is None:
        _NC1 = build_nc1()
        _NC2 = build_nc2()
    return _NC1, _NC2


def _rope_tables(positions):
    """positions [B, S] int -> packed cos/sin tables [128, T] f32 in token
    order (b*S + t); rows [0:64] and [64:128] hold the same 64 freqs."""
    inv_freq = 1.0 / (ROPE_BASE ** (np.arange(0, HEAD_DIM, 2, dtype=np.float64)
                                    / HEAD_DIM))
    freqs = np.asarray(positions).reshape(T).astype(np.float64)[:, None] * inv_freq
    cos = np.cos(freqs).T.astype(np.float32)  # [64, T]
    sin = np.sin(freqs).T.astype(np.float32)
    cos2 = np.concatenate([cos, cos], axis=0)  # [128, T]
    sin2 = np.concatenate([sin, sin], axis=0)
    scale = np.float32(HEAD_DIM ** -0.5)
    return ((cos2 * scale).astype(np.float16), (sin2 * scale).astype(np.float16),
            cos2.astype(np.float16), sin2.astype(np.float16))


def prepare_inputs1(hidden_states, positions, W_pack):
    x = np.ascontiguousarray(np.asarray(hidden_states, dtype=np.float32)
                             .reshape(T, HIDDEN))
    # [p, blk, kc, t]: per-partition-contiguous blocks (cheap DMA descriptors)
    xT_blocks = np.ascontiguousarray(
        x.T.astype(np.float16).reshape(KC, 128, NBLK, M_B)
        .transpose(1, 2, 0, 3))

    cosq, sinq, cosk, sink = _rope_tables(positions)

    tri = (np.arange(128)[:, None] <= np.arange(128)[None, :]) \
        .astype(np.float16)

    in_maps = []
    for c in range(N_CORES):
        hs = [HPC * c + i for i in range(HPC)]
        wqk_blocks = np.empty((8, 128, KC, 128), dtype=np.float16)
        for ft in range(8):
            off = 0 if ft < 4 else HIDDEN
            h = hs[ft % 4]
            wsl = W_pack[off + h * 128: off + (h + 1) * 128, :]  # [128, 4096]
            wqk_blocks[ft] = wsl.reshape(128, KC, 128).transpose(2, 1, 0)
        wv_sl = np.concatenate(
            [W_pack[2 * HIDDEN + h * 128: 2 * HIDDEN + (h + 1) * 128, :]
             for h in hs], axis=0)  # [512, 4096]
        wv_blocks = np.ascontiguousarray(
            wv_sl.astype(np.float16).reshape(512, KC, 128).transpose(2, 1, 0))
        in_maps.append({
            "xT": xT_blocks,
            "wqk": np.ascontiguousarray(wqk_blocks),
            "wv": wv_blocks,
            "cosq": cosq, "sinq": sinq, "cosk": cosk, "sink": sink,
            "trimask": tri,
            "onesc": np.ones((128, 1), dtype=np.float16),
            "onesr": np.ones((1, 128), dtype=np.float16),
        })
    return in_maps


def prepare_inputs2(res1, W_o):
    """Host-side shard permutation (the "AllToAll"): pure gather, no math."""
    wo_blocks = np.ascontiguousarray(
        np.ascontiguousarray(np.asarray(W_o, dtype=np.float32).T
                             .astype(np.float16))
        .reshape(KC, 128, 32, 128).transpose(2, 1, 0, 3))
    in_maps2 = []
    for j in range(N_CORES):
        yT = np.concatenate([res1.results[c]["attnT"][j] for c in range(N_CORES)],
                            axis=0)  # [4096, 512] feature-major, head order
        yTp = np.ascontiguousarray(
            yT.reshape(KC, 128, 512).transpose(1, 0, 2))  # [p, kc, t]
        in_maps2.append({"yT": yTp, "wo": wo_blocks})
    return in_maps2


def assemble(res2):
    out = np.empty((B, S, HIDDEN), dtype=np.float32)
    for c in range(N_CORES):
        bb, j = c // 4, c % 4
        out[bb, j * 512:(j + 1) * 512, :] = \
            res2.results[c]["out"].T.astype(np.float32)
    return out


def run(hidden_states, positions, W_pack, W_o, trace=False):
    nc1, nc2 = get_ncs()
    in_maps1 = prepare_inputs1(hidden_states, positions,
                               np.asarray(W_pack, dtype=np.float32))
    res1 = run_bass_kernel_spmd(nc1, in_maps1, list(range(N_CORES)),
                                trace=trace)
    in_maps2 = prepare_inputs2(res1, W_o)
    res2 = run_bass_kernel_spmd(nc2, in_maps2, list(range(N_CORES)),
                                trace=trace)
    return assemble(res2), res1, res2


def kernel(hidden_states, positions, W_pack, W_o):
    out, _, _ = run(hidden_states, positions, W_pack, W_o)
    return out
